# revision 60
# baseline (speedup 1.0000x reference)
"""MultiHeadAttention Trainium2 kernel, 8-way sharded (batch x head-group).

Sharding: core = 4*b + g  (b in {0,1} batch, g in {0..3} head-group of 4 heads).
Host pre-transposes x to bf16 tiles and pre-formats weights so the device does
zero transposes and zero dtype staging. Each core:
  - projects its batch's x_q/x_k/x_v with its 4 heads' weight slices (bf16),
    biases folded into the PSUM->SBUF eviction on the DVE,
  - runs causal attention for its 4 heads in S^T layout (keys on partitions),
    softmax denominator folded into the PV matmul via an augmented ones
    column in V, causal masking via a DVE multiply on the diagonal corner,
    with the PV matmuls lagging the score matmuls by 3 k-tiles so the PE
    never waits on the Act-engine exp,
  - computes the partial output projection (row-parallel Wo slice), with the
    next block's Q/K projections emitted in between to keep the PE busy
    through the softmax-normalize,
  - ReduceScatters partials over its 4-core batch group; the last block's RS
    is split into 4 per-128-row chunks to shrink the exposed tail.
Host assembles the strips into [2, 2048, 1024].
"""
import sys

for _p in ("/opt/trn_rl_repo",):
    if _p not in sys.path:
        sys.path.insert(0, _p)

import numpy as np
import ml_dtypes

import concourse.bass as bass
import concourse.tile as tile
from concourse import bacc, mybir
from concourse.bass_utils import run_bass_kernel_spmd


def _install_ntff_hook_shim():
    """The agent container's antenv lacks axon_hooks; recreate it so
    run_bass_kernel_spmd(trace=True) can profile via the axon .so."""
    import types, contextlib, ctypes, os

    if "antenv.axon_hooks" in sys.modules:
        return
    mod = types.ModuleType("antenv.axon_hooks")
    _store = {"hook": None}
    mod.set_axon_ntff_profile_hook = lambda h: _store.__setitem__("hook", h)
    mod.get_axon_ntff_profile_hook = lambda: _store["hook"]
    sys.modules["antenv.axon_hooks"] = mod

    so_path = "/opt/axon/libaxon_pjrt.so"
    if not os.path.exists(so_path):
        return
    try:
        lib = ctypes.CDLL(so_path)
        if not hasattr(lib, "axon_start_nrt_profile"):
            return
        lib.axon_start_nrt_profile.argtypes = [
            ctypes.POINTER(ctypes.c_int64), ctypes.c_size_t]
        lib.axon_start_nrt_profile.restype = ctypes.c_int64
        lib.axon_stop_nrt_profile.argtypes = [ctypes.c_char_p]
        lib.axon_stop_nrt_profile.restype = ctypes.c_int64

        @contextlib.contextmanager
        def _hook(output_dir, device_ids):
            import jax
            jax.devices()
            if device_ids:
                ids = (ctypes.c_int64 * len(device_ids))(*device_ids)
                rc = lib.axon_start_nrt_profile(ids, len(device_ids))
            else:
                rc = lib.axon_start_nrt_profile(None, 0)
            if rc != 0:
                raise RuntimeError(f"axon_start_nrt_profile rc={rc}")
            try:
                yield
            finally:
                n = lib.axon_stop_nrt_profile(str(output_dir).encode())
                print(f"ntff profile: {n} file(s) written to {output_dir}")

        mod.set_axon_ntff_profile_hook(_hook)
    except Exception:
        pass


_install_ntff_hook_shim()

F32 = mybir.dt.float32
BF16 = mybir.dt.bfloat16
AF = mybir.ActivationFunctionType
ALU = mybir.AluOpType

B, S, D_EMB = 2, 2048, 1024
H, DH = 16, 64
HG = 4              # heads per core
DM_L = HG * DH      # 256 local mid dim
D_OUT = 1024
NCORES = 8
ST = S // 128       # 16 s-tiles
ET = D_EMB // 128   # 8 emb tiles
QC = 4              # q chunks of 512
SCALE = 1.0 / 8.0   # 1/sqrt(DH)
LAG = 2             # PV matmuls lag the score matmuls by this many kt-PAIRS

# augmented V layout: per head slice [v(64), one] -> PV output rows 0..63 = O,
# row 64 = softmax denominator (the ones column sums P over keys).
HOFF = [0, 65, 130, 195]
WV_AUG = 260


def _build():
    nc = bacc.Bacc(None, target_bir_lowering=False, num_devices=NCORES)

    # x^T tiles: row (qc*ET+ei)*128 + p holds emb ei*128+p, col = seq within qc
    xk = nc.declare_dram_parameter("xk", [QC * ET * 128, 512], BF16, isOutput=False)
    xq = nc.declare_dram_parameter("xq", [QC * ET * 128, 512], BF16, isOutput=False)
    xv = nc.declare_dram_parameter("xv", [QC * ET * 128, 512], BF16, isOutput=False)
    # weights pre-tiled: [128, ET*cols] with col block ei holding emb ei*128+p
    wq = nc.declare_dram_parameter("wq", [128, ET * DM_L], BF16, isOutput=False)
    wk = nc.declare_dram_parameter("wk", [128, ET * DM_L], BF16, isOutput=False)
    wv = nc.declare_dram_parameter("wv", [128, ET * WV_AUG], BF16, isOutput=False)
    wo = nc.declare_dram_parameter("wo", [128, 2 * D_OUT], BF16, isOutput=False)
    bq2 = nc.declare_dram_parameter("bq2", [128, 2], F32, isOutput=False)
    bk2 = nc.declare_dram_parameter("bk2", [128, 2], F32, isOutput=False)
    bv = nc.declare_dram_parameter("bv", [WV_AUG], F32, isOutput=False)
    mtri = nc.declare_dram_parameter("mtri", [128, 128], BF16, isOutput=False)
    # per-chunk RS outputs (bf16, bias-free); host adds bo + casts. Chunk c
    # covers q rows [c*256, c*256+256) of this core's batch; the core keeps
    # its group-rank's 64-row quarter. The last block (si 12..15) uses per-si
    # 128-row chunks so its tail ReduceScatters pipeline instead of
    # serializing after compute ends.
    outs = [nc.declare_dram_parameter(f"o{c}", [64, D_OUT], BF16, isOutput=True)
            for c in range(6)]
    outs3 = [nc.declare_dram_parameter(f"o3_{j}", [32, D_OUT], BF16, isOutput=True)
             for j in range(4)]

    with tile.TileContext(nc) as tc:
        _emit(nc, tc, xk.ap(), xq.ap(), xv.ap(), wq.ap(), wk.ap(), wv.ap(),
              wo.ap(), bq2.ap(), bk2.ap(), bv.ap(), mtri.ap(),
              [o.ap() for o in outs], [o.ap() for o in outs3])
    nc.compile()
    return nc


def _emit(nc, tc, xk, xq, xv, wq, wk, wv, wo, bq2, bk2, bv, mtri, outs, outs3):
    from contextlib import ExitStack

    ctx = ExitStack()
    consts = ctx.enter_context(tc.tile_pool(name="consts", bufs=1))
    wpool = ctx.enter_context(tc.tile_pool(name="wpool", bufs=1))
    persist = ctx.enter_context(tc.tile_pool(name="persist", bufs=1))
    xpool = ctx.enter_context(tc.tile_pool(name="xpool", bufs=2))
    ptp = ctx.enter_context(tc.tile_pool(name="ptp", bufs=6))
    smallp = ctx.enter_context(tc.tile_pool(name="smallp", bufs=2))
    outp = ctx.enter_context(tc.tile_pool(name="outp", bufs=3))
    # PSUM: 2x 2-bank score tiles (kt-pair fused exp) + 2 single-bank tiles
    # for projection/output-projection/warmup + 2 PV accumulators = 8 banks.
    psr2 = ctx.enter_context(tc.tile_pool(name="psr2", bufs=2, space="PSUM"))
    psf = ctx.enter_context(tc.tile_pool(name="psf", bufs=2, space="PSUM"))
    pop = ctx.enter_context(tc.tile_pool(name="pop", bufs=1, space="PSUM"))
    dram = ctx.enter_context(tc.tile_pool(name="dram", bufs=1, space="DRAM"))

    # ---- constants (small, on the sync DMA queue ahead of the weights) ----
    bq_sb = consts.tile([128, 2], F32)
    nc.sync.dma_start(bq_sb[:], bq2[:])
    bk_sb = consts.tile([128, 2], F32)
    nc.sync.dma_start(bk_sb[:], bk2[:])
    mtri_sb = consts.tile([128, 128], BF16)
    nc.sync.dma_start(mtri_sb[:], mtri[:])
    # broadcast on the gpsimd queue (parallel to the sync queue)
    bv_bc = consts.tile([128, WV_AUG], F32)
    bv_bcast_ap = bass.AP(tensor=bv.tensor, offset=bv.offset, ap=[[0, 128], [1, WV_AUG]])
    nc.gpsimd.dma_start(out=bv_bc[:], in_=bv_bcast_ap)

    # preload the exp table early (first ACTIVATE triggers the table DMA)
    dummy_f32 = consts.tile([1, 16], F32)
    nc.vector.memset(dummy_f32[:], 0.0)
    dummy_o = consts.tile([1, 16], F32)
    nc.scalar.activation(out=dummy_o[:], in_=dummy_f32[:], func=AF.Exp, scale=1.0)

    # ---- persistent projection outputs ----
    qT = [persist.tile([128, S], BF16, name=f"qT{i}") for i in range(2)]
    kT = [persist.tile([128, S], BF16, name=f"kT{i}") for i in range(2)]
    v_sb = persist.tile([128, ST, WV_AUG], BF16)
    s1T = [persist.tile([128, S], BF16, name=f"s1T{i}") for i in range(2)]
    cc_in = [dram.tile([256, D_OUT], BF16, name=f"cc_in{c}") for c in range(6)]
    cc_out = [dram.tile([64, D_OUT], BF16, name=f"cc_out{c}") for c in range(6)]
    cc_in3 = [dram.tile([128, D_OUT], BF16, name=f"cc_in3_{j}") for j in range(4)]
    cc_out3 = [dram.tile([32, D_OUT], BF16, name=f"cc_out3_{j}") for j in range(4)]

    xsrc = {"k": xk, "q": xq, "v": xv}

    def load_x(key, qc, eng):
        """DMA the 8 pre-transposed bf16 [128,512] ei-tiles for q-chunk qc."""
        t = xpool.tile([128, ET, 512], BF16, tag=f"x{key}", name=f"x{key}{qc}")
        for ei in range(ET):
            r0 = (qc * ET + ei) * 128
            eng.dma_start(t[:, ei, :], xsrc[key][r0:r0 + 128, :])
        return t

    # ---- weights + first x block, per-ei interleaved across four DMA
    # queues so the very first projection matmuls are never DMA-paced ----
    xt = {}
    wk_sb = wpool.tile([128, ET, DM_L], BF16)
    wq_sb = wpool.tile([128, ET, DM_L], BF16)
    xt[("k", 0)] = xpool.tile([128, ET, 512], BF16, tag="xk", name="xk0")
    xt[("q", 0)] = xpool.tile([128, ET, 512], BF16, tag="xq", name="xq0")
    for ei in range(ET):
        qk = nc.sync if ei % 2 == 0 else nc.scalar
        qk.dma_start(wk_sb[:, ei, :], wk[:, ei * DM_L:(ei + 1) * DM_L])
        qk.dma_start(xt[("k", 0)][:, ei, :], xk[ei * 128:(ei + 1) * 128, :])
        nc.gpsimd.dma_start(wq_sb[:, ei, :], wq[:, ei * DM_L:(ei + 1) * DM_L])
        nc.gpsimd.dma_start(xt[("q", 0)][:, ei, :], xq[ei * 128:(ei + 1) * 128, :])
    wv_sb = wpool.tile([128, ET, WV_AUG], BF16)
    nc.sync.dma_start(wv_sb[:], wv[:])
    xt[("v", 0)] = load_x("v", 0, nc.gpsimd)
    wo_sb = wpool.tile([128, 2, D_OUT], BF16)
    nc.sync.dma_start(wo_sb[:], wo[:])

    # ---- PE warm-up while the first weight/x DMAs land (HAM unthrottle) ----
    warm_sb = consts.tile([128, 128], BF16)
    nc.vector.memset(warm_sb[:], 0.0)
    for w in range(2):
        warm_ps = psf.tile([128, 512], F32, tag="ps", name="warm")
        for _ in range(8):
            nc.tensor.matmul(
                warm_ps[:, 0:128], lhsT=warm_sb[:], rhs=warm_sb[:],
                start=True, stop=True,
            )

    def proj_qk_half(t, w_sb, b_sb, dst, qc, c2):
        pp = psf.tile([128, 512], F32, tag="ps", name="pp")
        for ei in range(ET):
            nc.tensor.matmul(
                pp[:, 0:512],
                lhsT=w_sb[:, ei, c2 * 128:(c2 + 1) * 128],
                rhs=t[:, ei, :],
                start=(ei == 0), stop=(ei == ET - 1),
            )
        nc.vector.tensor_scalar(
            out=dst[c2][:, qc * 512:(qc + 1) * 512], in0=pp[:, 0:512],
            scalar1=b_sb[:, c2:c2 + 1], scalar2=None, op0=ALU.add,
        )

    def proj_v_one(t, qc, r):
        si = 4 * qc + r
        pv = psf.tile([128, 512], F32, tag="ps", name="pv")
        for ei in range(ET):
            nc.tensor.matmul(
                pv[:, 0:WV_AUG],
                lhsT=t[:, ei, r * 128:(r + 1) * 128],
                rhs=wv_sb[:, ei, :],
                start=(ei == 0), stop=(ei == ET - 1),
            )
        nc.vector.tensor_tensor(
            out=v_sb[:, si, :], in0=pv[:, 0:WV_AUG], in1=bv_bc[:], op=ALU.add,
        )

    for _c2 in range(2):
        proj_qk_half(xt[("k", 0)], wk_sb, bk_sb, kT, 0, _c2)
    for _c2 in range(2):
        proj_qk_half(xt[("q", 0)], wq_sb, bq_sb, qT, 0, _c2)
    for _r in range(4):
        proj_v_one(xt[("v", 0)], 0, _r)

    def outproj_si(si):
        """Output projection for one 128-row block; after the odd block of
        each 256-row chunk, fire that chunk's ReduceScatter immediately so
        the CC stream runs concurrently with the rest of the kernel."""
        c = si // 2
        ob = outp.tile([128, D_OUT], BF16, tag="ob")
        for half in range(2):
            pp = psf.tile([128, 512], F32, tag="ps", name="op")
            for c2 in range(2):
                nc.tensor.matmul(
                    pp[:, 0:512],
                    lhsT=s1T[c2][:, si * 128:(si + 1) * 128],
                    rhs=wo_sb[:, c2, half * 512:(half + 1) * 512],
                    start=(c2 == 0), stop=(c2 == 1),
                )
            nc.vector.tensor_copy(
                out=ob[:, half * 512:(half + 1) * 512], in_=pp[:, 0:512]
            )
        if si >= 12:
            j = si - 12
            nc.gpsimd.dma_start(cc_in3[j][:], ob[:])
            nc.gpsimd.collective_compute(
                "ReduceScatter", ALU.add, replica_groups=RG,
                ins=[cc_in3[j].opt()], outs=[cc_out3[j].opt()],
            )
        else:
            nc.gpsimd.dma_start(cc_in[c][(si % 2) * 128:(si % 2) * 128 + 128, :], ob[:])
            if si % 2 == 1:
                nc.gpsimd.collective_compute(
                    "ReduceScatter", ALU.add, replica_groups=RG,
                    ins=[cc_in[c].opt()], outs=[cc_out[c].opt()],
                )

    RG = [[0, 1, 2, 3], [4, 5, 6, 7]]

    class FillSched:
        """Spreads filler work items (projections for the next block, output
        projection + RS for the previous block) evenly across the attention
        kt-steps, so no engine sees a burst at block boundaries and the PE
        always has ready work while the Act-engine exp pipeline catches up."""

        def __init__(self, items, total_steps):
            self.items = items
            self.total = max(total_steps, 1)
            self.step_no = 0
            self.idx = 0

        def step(self):
            self.step_no += 1
            target = (self.step_no * len(self.items) + self.total - 1) // self.total
            while self.idx < min(target, len(self.items)):
                self.items[self.idx]()
                self.idx += 1

        def drain(self):
            while self.idx < len(self.items):
                self.items[self.idx]()
                self.idx += 1

    def normalize_head(qc, p, h, po_h):
        """O^T / den -> s1T (den = PV row 64 via the ones column). All on
        DVE/gpsimd so the Act engine stays a pure exp pipeline."""
        den = smallp.tile([1, 512], F32, tag="den")
        nc.vector.tensor_copy(out=den[:], in_=po_h[64:65, 0:512])
        den_bc = smallp.tile([64, 512], F32, tag="denbc")
        nc.gpsimd.partition_broadcast(den_bc[:], den[:])
        rec = smallp.tile([64, 512], F32, tag="rec")
        nc.vector.reciprocal_approx_fast(out=rec[:], in_=den_bc[:])
        if h % 2 == 0:
            nc.vector.tensor_tensor(
                out=s1T[p][0:64, qc * 512:(qc + 1) * 512],
                in0=po_h[0:64, 0:512], in1=rec[:], op=ALU.mult,
            )
        else:
            # DVE lanes can't cross partitions: normalize at base 0,
            # then DMA the bf16 block to partitions 64..127 of s1T.
            tmp = smallp.tile([64, 512], BF16, tag="otmp")
            nc.vector.tensor_tensor(
                out=tmp[:], in0=po_h[0:64, 0:512], in1=rec[:], op=ALU.mult,
            )
            nc.gpsimd.dma_start(
                s1T[p][64:128, qc * 512:(qc + 1) * 512], tmp[:]
            )

    def attn_head(qc, p, h, fill):
        """Causal attention for one head of q-chunk qc. k-tiles are processed
        in pairs: both scores land in one 2-bank PSUM tile and a single
        [128,1024] exp converts them (fewer, larger Act instructions). PV
        matmuls lag the scores by LAG kt-pairs so the Act-engine exp is never
        on the PE critical path; heads run sequentially so the 2-tile score
        ring gives two full kt-pair steps of elasticity."""
        n_k = 4 * qc + 4
        base = 64 * (h % 2)
        po = pop.tile([128, 512], F32, tag=f"po{h % 2}", name=f"po{h}")
        pend = []

        def q0_of(kt):
            return 128 * (kt - 4 * qc) if kt >= 4 * qc else 0

        def pv_pop():
            ktp, pt2 = pend.pop(0)
            for j in range(2):
                kt = 2 * ktp + j
                q0 = q0_of(kt)
                nc.tensor.matmul(
                    po[0:65, q0:512],
                    lhsT=v_sb[:, kt, HOFF[h]:HOFF[h] + 65],
                    rhs=pt2[:, j, q0:512],
                    start=(kt == 0), stop=(kt == n_k - 1),
                )

        for ktp in range(n_k // 2):
            st2 = psr2.tile([128, 2, 512], F32, tag="st2", name=f"st{h}")
            for j in range(2):
                kt = 2 * ktp + j
                q0 = q0_of(kt)
                nc.tensor.matmul(
                    st2[:, j, q0:512],
                    lhsT=kT[p][base:base + 64, kt * 128:(kt + 1) * 128],
                    rhs=qT[p][base:base + 64, qc * 512 + q0:(qc + 1) * 512],
                    start=True, stop=True,
                )
            pt2 = ptp.tile([128, 2, 512], BF16, tag="pt", name=f"pt{h}")
            # one exp over both kt slots; untouched PSUM columns below the
            # diagonal offset produce garbage that no PV ever reads.
            nc.scalar.activation(
                out=pt2[:, :, :], in_=st2[:, :, :], func=AF.Exp, scale=SCALE,
            )
            for j in range(2):
                kt = 2 * ktp + j
                if kt >= 4 * qc:
                    q0 = q0_of(kt)
                    nc.vector.tensor_tensor(
                        out=pt2[:, j, q0:q0 + 128],
                        in0=pt2[:, j, q0:q0 + 128], in1=mtri_sb[:],
                        op=ALU.mult,
                    )
            pend.append((ktp, pt2))
            if len(pend) > LAG:
                pv_pop()
            fill.step()
        while pend:
            pv_pop()
        normalize_head(qc, p, h, po)

    # ---- fused pipeline: filler items spread across the attention steps ----
    for qc in range(QC):
        items = []
        if qc >= 1:
            prev = qc - 1
            items += [lambda si=si: outproj_si(si)
                      for si in range(4 * prev, 4 * prev + 4)]
        if qc < 3:
            # prefetch next block's x tiles across the DMA queues
            xt[("k", qc + 1)] = load_x("k", qc + 1, nc.sync)
            xt[("q", qc + 1)] = load_x("q", qc + 1, nc.gpsimd)
            xt[("v", qc + 1)] = load_x("v", qc + 1, nc.sync)
            items += [lambda c2=c2, qn=qc + 1: proj_qk_half(
                xt[("k", qn)], wk_sb, bk_sb, kT, qn, c2) for c2 in range(2)]
            items += [lambda c2=c2, qn=qc + 1: proj_qk_half(
                xt[("q", qn)], wq_sb, bq_sb, qT, qn, c2) for c2 in range(2)]
            items += [lambda r=r, qn=qc + 1: proj_v_one(
                xt[("v", qn)], qn, r) for r in range(4)]
        # stretch the schedule past the kt-steps so ~25% of the filler is
        # still left to drain during the last head's normalize window (keeps
        # the PE fed through the block boundary).
        fill = FillSched(items, (8 * (qc + 1) * 4) // 3 + 1)
        for p in range(2):
            for h in (2 * p, 2 * p + 1):
                attn_head(qc, p, h, fill)
        fill.drain()

    # last block's output projection + RS chunks
    for si in range(12, 16):
        outproj_si(si)

    # DRAM->DRAM copies of the RS results to the IO tensors (collectives
    # can't write IO tensors directly). Sync queue is idle by now.
    for c in range(6):
        nc.sync.dma_start(outs[c][:], cc_out[c][:])
    for j in range(4):
        nc.sync.dma_start(outs3[j][:], cc_out3[j][:])

    ctx.close()


_NC_CACHE = None


def _get_nc():
    global _NC_CACHE
    if _NC_CACHE is None:
        _NC_CACHE = _build()
    return _NC_CACHE


def _tile_xT(x2d):
    """[2048, 1024] f32 -> bf16 x^T tiles [(qc*8+ei)*128+p, s]."""
    bf16 = ml_dtypes.bfloat16
    xT = np.ascontiguousarray(x2d.T).astype(bf16)          # [1024, 2048]
    t = xT.reshape(ET, 128, QC, 512).transpose(2, 0, 1, 3)  # [qc, ei, p, s]
    return np.ascontiguousarray(t.reshape(QC * ET * 128, 512))


def _tile_w(w2d, ncols):
    """[1024, ncols] f32 -> bf16 [128, ET*ncols] (col block ei)."""
    bf16 = ml_dtypes.bfloat16
    t = w2d.astype(bf16).reshape(ET, 128, ncols).transpose(1, 0, 2)
    return np.ascontiguousarray(t.reshape(128, ET * ncols))


def _make_in_maps(x_q, x_k, x_v, Wq, bq, Wk, bk, Wv, bv, Wo, bo):
    f32 = np.float32
    bf16 = ml_dtypes.bfloat16
    mtri_np = np.triu(np.ones((128, 128), f32)).astype(bf16)

    # per-batch x^T tiles (shared across the 4 cores of each batch group)
    xb = {}
    for b in range(B):
        xb[(b, "q")] = _tile_xT(np.asarray(x_q[b], f32))
        xb[(b, "k")] = _tile_xT(np.asarray(x_k[b], f32))
        xb[(b, "v")] = _tile_xT(np.asarray(x_v[b], f32))

    in_maps = []
    for core in range(NCORES):
        b, g = core // 4, core % 4
        sl = slice(g * DM_L, (g + 1) * DM_L)
        # augmented V weight/bias
        wv_aug = np.zeros((D_EMB, WV_AUG), f32)
        bv_aug = np.zeros((WV_AUG,), f32)
        for h in range(HG):
            gh = g * HG + h
            o = HOFF[h]
            wv_aug[:, o:o + 64] = Wv[:, gh * DH:(gh + 1) * DH]
            bv_aug[o:o + 64] = bv[gh * DH:(gh + 1) * DH]
            bv_aug[o + 64] = 1.0
        wo_t = np.ascontiguousarray(
            Wo[sl, :].astype(bf16).reshape(2, 128, D_OUT)
            .transpose(1, 0, 2).reshape(128, 2 * D_OUT))
        in_maps.append({
            "xq": xb[(b, "q")],
            "xk": xb[(b, "k")],
            "xv": xb[(b, "v")],
            "wq": _tile_w(np.asarray(Wq[:, sl], f32), DM_L),
            "wk": _tile_w(np.asarray(Wk[:, sl], f32), DM_L),
            "wv": _tile_w(wv_aug, WV_AUG),
            "wo": wo_t,
            "bq2": np.ascontiguousarray(bq[sl].reshape(2, 128).T, dtype=f32),
            "bk2": np.ascontiguousarray(bk[sl].reshape(2, 128).T, dtype=f32),
            "bv": bv_aug,
            "mtri": mtri_np,
        })
    return in_maps


def run(inputs, trace=False, trace_kwargs=None):
    """Run on 8 NeuronCores. Returns (output [2,2048,1024] f32, BassKernelResults)."""
    inputs = {k: np.asarray(v) for k, v in inputs.items()}
    nc = _get_nc()
    in_maps = _make_in_maps(
        inputs["x_q"], inputs["x_k"], inputs["x_v"],
        inputs["Wq"], inputs["bq"], inputs["Wk"], inputs["bk"],
        inputs["Wv"], inputs["bv"], inputs["Wo"], inputs["bo"],
    )
    kwargs = {}
    if trace:
        kwargs["trace"] = True
        if trace_kwargs:
            kwargs.update(trace_kwargs)
    res = run_bass_kernel_spmd(nc, in_maps, core_ids=list(range(NCORES)), **kwargs)
    bo_f = np.asarray(inputs["bo"], np.float32)
    out_full = np.empty((B, S, D_OUT), np.float32)
    for core in range(NCORES):
        b, g = core // 4, core % 4
        r = res.results[core]
        for c in range(6):
            out_full[b, c * 256 + g * 64:c * 256 + (g + 1) * 64, :] = \
                r[f"o{c}"].astype(np.float32)
        for j, si in enumerate(range(12, 16)):
            out_full[b, si * 128 + g * 32:si * 128 + (g + 1) * 32, :] = \
                r[f"o3_{j}"].astype(np.float32)
    out_full += bo_f
    return out_full, res


def kernel(**inputs) -> np.ndarray:
    out, _ = run(inputs, trace=False)
    return out
